# revision 2
# baseline (speedup 1.0000x reference)
"""HSTU attention (B=2, L=2048, D=1024, H=16) on 8 TRN2 NeuronCores.

Sharding: batch (2) x head-group (4 heads, 256 features) -> 8 cores.

Key optimization vs the dense baseline: keys are permuted per batch on the
host (valid keys only, prompt tokens first, then item tokens by position).
Prompt keys are bidirectionally visible and item keys causally visible, so
in sorted order each 512-query chunk ic needs only a PREFIX of key chunks
(n_total[ic] of them); chunks past the causal staircase are skipped
entirely.  Slot classification (dense prefix / masked suffix) is shared
SPMD-wide: n_dense = min over batches of the fully-allowed prefix,
n_total = max over batches of the needed prefix; per-batch differences are
absorbed in the per-core dmask data (all-zero or all-NEG tiles).

Per (ec, ic, slot): 2 score matmuls write one [128,1024] 2-bank PSUM pair,
masked slots get a DVE/Pool broadcast-add of a host dmask, ONE fused
activation computes exp of the pair, 2 AV matmuls accumulate output +
rowsums (ones-column trick in the V layout).  Projections (bf16 operands,
DMA-landed, no rounding copies) are interleaved into the attention stream
as PE filler work so the PE never waits on the Activation engine.
"""

import sys

for _p in ("/opt/trn_rl_repo", "/root/.axon_site/_ro/trn_rl_repo"):
    if _p not in sys.path:
        sys.path.insert(0, _p)

import numpy as np
import ml_dtypes

import concourse.bass as bass  # noqa: F401
import concourse.mybir as mybir
import concourse.tile as tile
from concourse import bacc
from concourse.bass_utils import run_bass_kernel_spmd

F32 = mybir.dt.float32
F32R = mybir.dt.float32r
BF16 = mybir.dt.bfloat16
EXP = mybir.ActivationFunctionType.Exp
BF = ml_dtypes.bfloat16

B, L, D, H = 2, 2048, 1024, 16
DK = D // H          # 64
HPC = 4              # heads per core
E = HPC * DK         # 256 features per core
NDC = D // 128       # 8 contraction chunks for projections
NIC = L // 512       # 4 query chunks
NEG = -10000.0

_cache = {}


# ---------------------------------------------------------------- planning

def _plan(token_types, seq_lens):
    """Per-batch sorted-key layout + shared SPMD slot structure."""
    jr = np.arange(L)
    per_b = []
    for b in range(B):
        s = int(seq_lens[b])
        prompt = np.asarray(token_types[b]) < 3
        valid = jr < s
        pj = jr[prompt & valid]
        ij = jr[(~prompt) & valid]
        perm = np.concatenate([pj, ij])
        per_b.append((perm, len(pj), len(ij)))

    NCH = max(-(-len(p[0]) // 128) for p in per_b)
    LK = NCH * 128

    # chunk classification per batch: kinds/origj padded to LK
    Ts, Ds = [], []
    batches = []
    for perm, npj, nij in per_b:
        nval = npj + nij
        kind = np.full(LK, 2, np.int8)          # 2 = dummy
        kind[:npj] = 0                           # prompt
        kind[npj:nval] = 1                       # item
        oj = np.full(LK, -1, np.int64)
        oj[:nval] = perm
        T = np.zeros(NIC, np.int64)
        Dn = np.zeros(NIC, np.int64)
        for ic in range(NIC):
            ihi = ic * 512 + 511
            ilo = ic * 512
            Tb = 0
            Db = 0
            run_dense = True
            for c in range(NCH):
                k = kind[c * 128:(c + 1) * 128]
                j = oj[c * 128:(c + 1) * 128]
                item = k == 1
                allowed_any = (k == 0).any() or (item & (j <= ihi)).any()
                disallowed_any = (k == 2).any() or (item & (j > ilo)).any()
                if allowed_any:
                    Tb = c + 1
                if disallowed_any:
                    run_dense = False
                if run_dense:
                    Db = c + 1
            T[ic] = Tb
            Dn[ic] = min(Db, Tb)
        Ts.append(T)
        Ds.append(Dn)
        batches.append((perm, kind, oj))

    n_total = tuple(int(max(Ts[b][ic] for b in range(B))) for ic in range(NIC))
    n_dense = tuple(int(min(Ds[b][ic] for b in range(B))) for ic in range(NIC))

    # Elementwise-mask slots: chunks where the causal staircase crosses the
    # query window for SOME batch (an item key with ilo < j <= ihi).  All
    # other masked slots have row-uniform masks -> free per-partition bias.
    elo, ehi = [], []
    for ic in range(NIC):
        ilo, ihi = ic * 512, ic * 512 + 511
        lo, hi = n_total[ic], n_dense[ic]
        for _, kind, oj in batches:
            for c in range(n_dense[ic], n_total[ic]):
                k = kind[c * 128:(c + 1) * 128]
                j = oj[c * 128:(c + 1) * 128]
                if ((k == 1) & (j > ilo) & (j <= ihi)).any():
                    lo = min(lo, c)
                    hi = max(hi, c + 1)
        if lo >= hi:
            lo = hi = n_dense[ic]
        elo.append(lo)
        ehi.append(hi)
    return NCH, n_total, n_dense, tuple(elo), tuple(ehi), batches


def _masks_for_batch(n_total, n_dense, elo, ehi, kind, oj):
    """Elementwise dmask tiles [NET,128,512] + per-row bias columns [128,NBT]."""
    tiles = []
    bias_cols = []
    for ic in range(NIC):
        iw = np.arange(ic * 512, ic * 512 + 512)
        ihi = ic * 512 + 511
        for c in range(n_dense[ic], n_total[ic]):
            k = kind[c * 128:(c + 1) * 128]
            j = oj[c * 128:(c + 1) * 128]
            if elo[ic] <= c < ehi[ic]:
                allowed = (k == 0)[:, None] | (
                    ((k == 1)[:, None]) & (j[:, None] <= iw[None, :])
                )
                tiles.append(np.where(allowed, 1.0, 0.0).astype(BF))
            else:
                # row-uniform over the window: allowed iff prompt or j<=ilo
                row_ok = (k == 0) | ((k == 1) & (j <= ihi))
                # must be uniform: either j<=ilo (all allowed) or j>ihi
                bias_cols.append(np.where(row_ok, 0.0, NEG).astype(np.float32))
    dmk = (np.stack(tiles) if tiles else np.zeros((0, 128, 512), BF))
    if bias_cols:
        bias = np.stack(bias_cols, axis=1).astype(np.float32)
    else:
        bias = np.zeros((128, 0), np.float32)
    return dmk, bias


# ---------------------------------------------------------------- build

def _build(NCH, n_total, n_dense, elo, ehi):
    LK = NCH * 128
    n_elem = tuple(ehi[i] - elo[i] for i in range(NIC))
    NET = sum(n_elem)
    NBT = sum(n_total[i] - n_dense[i] - n_elem[i] for i in range(NIC))
    NCP = -(-LK // 512)                     # 512-col chunks of sorted keys

    nc = bacc.Bacc("TRN2", target_bir_lowering=False, debug=False)

    xt = nc.dram_tensor("xt", [128, NIC, NDC * 512], BF16,
                        kind="ExternalInput").ap()
    xp = nc.dram_tensor("xp", [128, NCP, NDC * 512], BF16,
                        kind="ExternalInput").ap()
    wq = nc.dram_tensor("wq", [128, NDC, E], BF16, kind="ExternalInput").ap()
    wk = nc.dram_tensor("wk", [128, NDC, E], BF16, kind="ExternalInput").ap()
    wv = nc.dram_tensor("wv", [128, NDC, E], BF16, kind="ExternalInput").ap()
    wu = nc.dram_tensor("wu", [128, NDC, E], BF16, kind="ExternalInput").ap()
    wo = nc.dram_tensor("wo", [E, D], F32R, kind="ExternalInput").ap()
    dmask = nc.dram_tensor("dmask", [max(NET, 1), 128, 512], BF16,
                           kind="ExternalInput").ap()
    biasd = nc.dram_tensor("biasd", [128, max(NBT, 1)], F32,
                           kind="ExternalInput").ap()
    out = nc.dram_tensor("out", [L, D], F32, kind="ExternalOutput").ap()
    out2 = nc.dram_tensor("out2", [512, D], F32, kind="ExternalOutput").ap()
    out2 = nc.dram_tensor("out2", [512, D], F32, kind="ExternalOutput").ap()

    NMmax = max(max(n_elem), 1)
    # index maps: per (ic, slot) -> dmask column (within ic tile) or bias col
    bias_base = []
    acc = 0
    for ic in range(NIC):
        bias_base.append(acc)
        acc += n_total[ic] - n_dense[ic] - n_elem[ic]

    with tile.TileContext(nc) as tc:
        with tc.tile_pool(name="persist", bufs=1) as persist, \
             tc.tile_pool(name="pp", bufs=2, space="PSUM") as pp, \
             tc.tile_pool(name="ps", bufs=2, space="PSUM") as ps, \
             tc.tile_pool(name="po", bufs=2, space="PSUM") as po, \
             tc.tile_pool(name="xtl", bufs=2) as xtlp, \
             tc.tile_pool(name="epool", bufs=5) as epool, \
             tc.tile_pool(name="erpool", bufs=3) as erpool, \
             tc.tile_pool(name="dpool", bufs=2) as dpool, \
             tc.tile_pool(name="gst", bufs=2) as gst, \
             tc.tile_pool(name="rpool", bufs=2) as rpool, \
             tc.tile_pool(name="ost", bufs=2) as ost:

            qt = [persist.tile([128, L], F32R, tag=f"qt{i}", name=f"qt{i}")
                  for i in range(2)]
            ut = [persist.tile([128, L], F32R, tag=f"ut{i}", name=f"ut{i}")
                  for i in range(2)]
            g = [persist.tile([128, L], F32R, tag=f"g{i}", name=f"g{i}")
                 for i in range(2)]
            kt = [persist.tile([128, LK], F32R, tag=f"kt{i}", name=f"kt{i}")
                  for i in range(2)]
            # v layout per key chunk: 4 slots x 128: head h at slot h cols
            # 0:64; ones at flat cols 64 and 320; zeros at 65:128, 321:384.
            v = persist.tile([128, NCH, 512], F32R, tag="v")
            wqt = persist.tile([128, NDC, E], BF16, tag="wqt")
            wkt = persist.tile([128, NDC, E], BF16, tag="wkt")
            wvt = persist.tile([128, NDC, E], BF16, tag="wvt")
            wut = persist.tile([128, NDC, E], BF16, tag="wut")
            wo_r = [persist.tile([128, D], F32R, tag=f"wor{i}", name=f"wor{i}")
                    for i in range(2)]
            onesf = persist.tile([128, 128], F32, tag="onesf")
            ones_r = persist.tile([128, 128], F32R, tag="onesr")
            zerof = persist.tile([128, 63], F32, tag="zerof")
            bias_t = persist.tile([128, max(NBT, 1)], F32, tag="biast")
            nc.sync.dma_start(out=bias_t, in_=biasd)

            # ---- upfront DMAs (K weights + first x chunk first) ----
            nc.sync.dma_start(out=wkt, in_=wk)

            xpl_tiles = []

            def load_xp(ch, split=False):
                t = xtlp.tile([128, NDC, 512], BF16, tag="xp", name="xpl")
                src3 = xp[:, ch, :].rearrange("p (dc n) -> p dc n", n=512)
                if split:
                    nc.sync.dma_start(out=t[:, 0:4, :], in_=src3[:, 0:4, :])
                    nc.sync.dma_start(out=t[:, 4:8, :], in_=src3[:, 4:8, :])
                else:
                    nc.sync.dma_start(out=t, in_=src3)
                return t

            xpl_tiles.append(load_xp(0, split=True))
            nc.sync.dma_start(out=wvt, in_=wv)
            if NCP > 1:
                xpl_tiles.append(load_xp(1))

            nc.sync.dma_start(out=wqt, in_=wq)
            nc.sync.dma_start(out=wut, in_=wu)
            for ec in range(2):
                nc.sync.dma_start(out=wo_r[ec], in_=wo[ec * 128:(ec + 1) * 128, :])

            xtl_tiles = {}

            def load_xt(ic):
                t = xtlp.tile([128, NDC, 512], BF16, tag="xt", name="xtl")
                nc.sync.dma_start(
                    out=t, in_=xt[:, ic, :].rearrange("p (dc n) -> p dc n",
                                                      n=512))
                xtl_tiles[ic] = t

            load_xt(0)

            nc.vector.memset(onesf, 1.0)
            nc.vector.memset(zerof, 0.0)
            nc.vector.tensor_copy(ones_r, onesf)
            v3 = v  # [128, NCH, 512]
            nc.vector.tensor_copy(v3[:, :, 64:65], ones_r[:, 0:NCH].unsqueeze(2))
            nc.vector.tensor_copy(v3[:, :, 320:321], ones_r[:, 0:NCH].unsqueeze(2))
            nc.vector.tensor_copy(
                v3[:, :, 65:128], zerof.unsqueeze(1).broadcast_to([128, NCH, 63]))
            nc.vector.tensor_copy(
                v3[:, :, 321:384], zerof.unsqueeze(1).broadcast_to([128, NCH, 63]))

            dm_tiles = {}

            def load_dmask(ic):
                if n_elem[ic] == 0:
                    return
                m0 = sum(n_elem[:ic])
                t = dpool.tile([128, NMmax, 512], BF16, tag="dm", name="dmt")
                nc.sync.dma_start(
                    out=t[:, 0:n_elem[ic], :],
                    in_=dmask[m0:m0 + n_elem[ic]].rearrange("m p q -> p m q"),
                )
                dm_tiles[ic] = t

            load_dmask(0)

            # ---- phase A: K and V projections from permuted x ----
            # (the tail: K last chunk + V lc >= 4*(NCP-2)+1 becomes filler
            #  work inside the first attention block)
            def k_chunk(ch, ec):
                w = min(512, LK - ch * 512)
                xc = xpl_tiles[ch]
                p = pp.tile([128, 512], F32, tag="pp", name="pk")
                for dc in range(NDC):
                    nc.tensor.matmul(
                        p[:, 0:w],
                        wkt[:, dc, ec * 128:(ec + 1) * 128],
                        xc[:, dc, 0:w],
                        start=(dc == 0), stop=(dc == NDC - 1),
                    )
                nc.vector.tensor_copy(
                    kt[ec][:, ch * 512:ch * 512 + w], p[:, 0:w])

            def v_chunk(lc):
                ch, ii = lc // 4, lc % 4
                xc = xpl_tiles[ch]
                p = pp.tile([128, 512], F32, tag="pp", name="pv")
                for dc in range(NDC):
                    nc.tensor.matmul(
                        p[:, 0:E],
                        xc[:, dc, ii * 128:(ii + 1) * 128],
                        wvt[:, dc, :],
                        start=(dc == 0), stop=(dc == NDC - 1),
                    )
                pv = p[:, 0:E].rearrange("p (h n) -> p h n", n=64)
                vd = v3[:, lc, :].rearrange("p (s n) -> p s n", n=128)
                nc.vector.tensor_copy(vd[:, 0:4, 0:64], pv)

            # enough K/V for the first attention block (chunks < n_total[0])
            lc_cut = min(n_total[0], NCH)
            for ch in range(NCP - 1):
                for ec in range(2):
                    k_chunk(ch, ec)
                for ii in range(4):
                    lc = ch * 4 + ii
                    if lc < lc_cut:
                        v_chunk(lc)
                if ch + 2 < NCP:
                    xpl_tiles.append(load_xp(ch + 2))
            deferred_kv = []
            for ec in range(2):
                deferred_kv.append(lambda ec=ec: k_chunk(NCP - 1, ec))
            for lc in range(lc_cut, NCH):
                deferred_kv.append(lambda lc=lc: v_chunk(lc))

            def proj_qu_units(nm, ic, ec):
                """Two filler units (4 matmuls each) forming one Q/U chain."""
                wt, dst = (wqt, qt) if nm == "q" else (wut, ut)
                isl = slice(ic * 512, (ic + 1) * 512)
                st = {}

                def half(lo, hi):
                    def run():
                        if lo == 0:
                            st["p"] = pp.tile([128, 512], F32, tag="pp",
                                              name="pq")
                        p = st["p"]
                        xc = xtl_tiles[ic]
                        for dc in range(lo, hi):
                            nc.tensor.matmul(
                                p,
                                wt[:, dc, ec * 128:(ec + 1) * 128],
                                xc[:, dc, :],
                                start=(dc == 0), stop=(dc == NDC - 1),
                            )
                        if hi == NDC:
                            nc.vector.tensor_copy(dst[ec][:, isl], p)
                    return run

                return [half(0, 4), half(4, NDC)]

            def proj_qu(nm, ic, ec):
                for u in proj_qu_units(nm, ic, ec):
                    u()

            # Q/U for ic0 (needed before attention starts)
            for ec in range(2):
                proj_qu("q", 0, ec)
            load_xt(1)
            for ec in range(2):
                proj_qu("u", 0, ec)

            # ---- phase B: attention with interleaved fillers ----
            fillers = []

            def pump(n=1):
                for _ in range(n):
                    if fillers:
                        fillers.pop(0)()

            ost_tiles = {}

            def wo_half_unit(lc, fc, ec2):
                # single-ec W_o partial for the last query block; host sums
                def run():
                    p = pp.tile([128, 512], F32, tag="pp", name="pwoh")
                    nc.tensor.matmul(
                        p,
                        g[ec2][:, lc * 128:(lc + 1) * 128],
                        wo_r[ec2][:, fc * 512:(fc + 1) * 512],
                        start=True, stop=True,
                    )
                    dst = out2 if ec2 == 0 else out
                    row = (lc - 12) * 128 if ec2 == 0 else lc * 128
                    st = ost.tile([128, 512], F32, tag="osh", name="osth")
                    if (lc + fc) % 2 == 0:
                        nc.scalar.copy(st, p)
                    else:
                        nc.vector.tensor_copy(st, p)
                    nc.sync.dma_start(
                        out=dst[row:row + 128, fc * 512:(fc + 1) * 512],
                        in_=st)
                return run

            def wo_half_unit(lc, fc, ec2):
                # last-block W_o: per-ec partials (host sums out2 into out)
                def run():
                    pool_, tag_ = (ps, "ps") if ec2 == 1 else (pp, "pp")
                    p = pool_.tile([128, 512], F32, tag=tag_, name="pwoh")
                    nc.tensor.matmul(
                        p,
                        g[ec2][:, lc * 128:(lc + 1) * 128],
                        wo_r[ec2][:, fc * 512:(fc + 1) * 512],
                        start=True, stop=True,
                    )
                    st = ost.tile([128, 512], F32, tag="osh", name="osth")
                    if ec2 == 1 and lc % 2 == 0:
                        nc.scalar.copy(st, p)
                    else:
                        nc.vector.tensor_copy(st, p)
                    dst = out2 if ec2 == 0 else out
                    row = (lc - 12) * 128 if ec2 == 0 else lc * 128
                    nc.sync.dma_start(
                        out=dst[row:row + 128, fc * 512:(fc + 1) * 512],
                        in_=st)
                return run

            def wo_unit(ic, lc, fc):
                def run():
                    pool_ = ps if ic == 3 else pp
                    tag_ = "ps" if ic == 3 else "pp"
                    p = pool_.tile([128, 512], F32, tag=tag_, name="pwo")
                    for ec2 in range(2):
                        nc.tensor.matmul(
                            p,
                            g[ec2][:, lc * 128:(lc + 1) * 128],
                            wo_r[ec2][:, fc * 512:(fc + 1) * 512],
                            start=(ec2 == 0), stop=(ec2 == 1),
                        )
                    if fc == 0:
                        ost_tiles[lc] = ost.tile([128, 1024], F32, tag="os",
                                                 name="ostile")
                    o = ost_tiles[lc]
                    if ic == 3 and lc % 2 == 0:
                        nc.scalar.copy(o[:, fc * 512:(fc + 1) * 512], p)
                    else:
                        nc.vector.tensor_copy(o[:, fc * 512:(fc + 1) * 512], p)
                    if ic == 3:
                        nc.sync.dma_start(
                            out=out[lc * 128:(lc + 1) * 128,
                                    fc * 512:(fc + 1) * 512],
                            in_=o[:, fc * 512:(fc + 1) * 512])
                    elif fc == 1:
                        nc.sync.dma_start(
                            out=out[lc * 128:(lc + 1) * 128, :], in_=o)
                return run

            def attn_block(ec, ic):
                isl = slice(ic * 512, (ic + 1) * 512)
                nT, nD = n_total[ic], n_dense[ic]
                oA = po.tile([128, 512], F32, tag="po", name="oA")
                oB = po.tile([128, 512], F32, tag="po", name="oB")
                pend = None  # (e_tile, ch, first)
                nbias = 0
                for k in range(nT):
                    pair = ps.tile([128, 1024], F32, tag="ps", name="spair")
                    nc.tensor.matmul(
                        pair[:, 0:512],
                        kt[ec][0:64, k * 128:(k + 1) * 128],
                        qt[ec][0:64, isl],
                        start=True, stop=True,
                    )
                    nc.tensor.matmul(
                        pair[:, 512:1024],
                        kt[ec][64:128, k * 128:(k + 1) * 128],
                        qt[ec][64:128, isl],
                        start=True, stop=True,
                    )
                    e = epool.tile([128, 1024], F32R, tag="e", name="e")
                    if elo[ic] <= k < ehi[ic]:
                        m = k - elo[ic]
                        # plain exp, then SBUF-only 0/1 mask multiply,
                        # halves split across DVE and Pool so each AV half
                        # waits only on its own engine
                        eraw = erpool.tile([128, 1024], F32R, tag="er",
                                           name="eraw")
                        nc.scalar.activation(eraw, pair, EXP)
                        dmh = dm_tiles[ic][:, m, :]
                        with nc.allow_low_precision(
                                reason="f32r rounding for matmul"):
                            nc.vector.tensor_mul(
                                e[:, 0:512], eraw[:, 0:512], dmh)
                            nc.gpsimd.tensor_mul(
                                e[:, 512:1024], eraw[:, 512:1024], dmh)
                    elif k >= nD:
                        bidx = bias_base[ic] + nbias
                        nbias += 1
                        nc.scalar.activation(
                            e, pair, EXP, bias=bias_t[:, bidx:bidx + 1],
                            scale=1.0)
                    else:
                        nc.scalar.activation(e, pair, EXP)
                    pump(3 if (pend is not None and
                               elo[ic] <= pend[1] < ehi[ic]) else 1)
                    if pend is not None:
                        _issue_av(ec, oA, oB, *pend, last=False)
                    pend = (e, k, k == 0)
                _issue_av(ec, oA, oB, *pend, last=True)
                return oA, oB

            def _issue_av(ec, oA, oB, e, ch, first, last):
                vb = 256 * ec
                nc.tensor.matmul(
                    oA[0:65, :], v3[:, ch, vb:vb + 65], e[:, 0:512],
                    start=first, stop=last,
                )
                nc.tensor.matmul(
                    oB, v3[:, ch, vb + 64:vb + 192], e[:, 512:1024],
                    start=first, stop=last,
                )

            def gate(ec, ic, oA, oB):
                """Recips + O*U on DVE (PSUM reads); the recip broadcast and
                final g muls run SBUF-only on the otherwise idle Pool."""
                isl = slice(ic * 512, (ic + 1) * 512)
                rec = rpool.tile([128, 512], F32R, tag="rec", name="rec")
                with nc.allow_low_precision(reason="f32r rounding for matmul"):
                    nc.vector.reciprocal(rec[64:65, :], oA[64:65, :])
                    nc.vector.reciprocal(rec[0:1, :], oB[0:1, :])
                t1 = gst.tile([128, 512], F32, tag="t1", name="t1")
                nc.vector.tensor_mul(t1[0:64, :], oA[0:64, :], ut[ec][0:64, isl])
                nc.vector.tensor_mul(
                    t1[64:128, :], oB[64:128, :], ut[ec][64:128, isl])
                def finish():
                    pA = pp.tile([128, 512], F32, tag="pp", name="pcA")
                    nc.tensor.matmul(
                        pA, ones_r[64:65, :], rec[64:65, :],
                        start=True, stop=True)
                    pB = pp.tile([128, 512], F32, tag="pp", name="pcB")
                    nc.tensor.matmul(
                        pB, ones_r[0:1, :], rec[0:1, :],
                        start=True, stop=True)
                    with nc.allow_low_precision(
                            reason="f32r rounding for matmul"):
                        nc.vector.tensor_mul(
                            g[ec][0:64, isl], t1[0:64, :], pA[0:64, :])
                        nc.vector.tensor_mul(
                            g[ec][64:128, isl], t1[64:128, :], pB[64:128, :])
                fillers.insert(0, finish)

            # filler assignment per block: projections early, W_o shifted to
            # the later (ACT-heavy, filler-poor) blocks
            # (icw, lc) pairs per block: wo(1) split across ic2/ic3
            wo_sched = {1: [(0, lc) for lc in range(0, 4)],
                        2: [(1, lc) for lc in range(4, 6)],
                        3: [(1, lc) for lc in range(6, 8)] +
                           [(2, lc) for lc in range(8, 12)]}
            for ic in range(NIC):
                if ic == 0:
                    fillers.extend(deferred_kv)
                if ic + 1 < NIC:
                    if ic + 2 < NIC:
                        load_xt(ic + 2)
                    for ec in range(2):
                        fillers.extend(proj_qu_units("q", ic + 1, ec))
                        fillers.extend(proj_qu_units("u", ic + 1, ec))
                for icw, lc in wo_sched.get(ic, []):
                    for fc in range(2):
                        fillers.append(wo_unit(icw, lc, fc))
                if ic + 1 < NIC:
                    load_dmask(ic + 1)
                for ec in range(2):
                    oA, oB = attn_block(ec, ic)
                    gate(ec, ic, oA, oB)

            while fillers:
                pump(1)
            for lc in range(12, 16):
                for fc in range(2):
                    wo_unit(3, lc, fc)()

    nc.compile()
    return nc


# ---------------------------------------------------------------- host

def _host_inputs(x, token_types, seq_lens, W_q, W_k, W_v, W_u, W_o, plan):
    NCH, n_total, n_dense, elo, ehi, batches = plan
    LK = NCH * 128
    x = np.asarray(x, dtype=np.float32)
    W_q = np.asarray(W_q, dtype=np.float32)
    W_k = np.asarray(W_k, dtype=np.float32)
    W_v = np.asarray(W_v, dtype=np.float32)
    W_u = np.asarray(W_u, dtype=np.float32)
    W_o = np.asarray(W_o, dtype=np.float32)

    per_batch = []
    for b in range(B):
        perm, kind, oj = batches[b]
        NCP = -(-LK // 512)
        xtb = np.ascontiguousarray(
            x[b].T.astype(BF).reshape(NDC, 128, NIC, 512)
            .transpose(1, 2, 0, 3).reshape(128, NIC, NDC * 512))
        xperm = np.zeros((NCP * 512, D), np.float32)
        xperm[:len(perm)] = x[b][perm]
        xpb = np.ascontiguousarray(
            xperm.T.astype(BF).reshape(NDC, 128, NCP, 512)
            .transpose(1, 2, 0, 3).reshape(128, NCP, NDC * 512))
        dmk, bias = _masks_for_batch(n_total, n_dense, elo, ehi, kind, oj)
        if dmk.shape[0] == 0:
            dmk = np.zeros((1, 128, 512), np.float32)
        if bias.shape[1] == 0:
            bias = np.zeros((128, 1), np.float32)
        per_batch.append((xtb, xpb, dmk, np.ascontiguousarray(bias)))

    def _warr(w):  # [E, D] slice -> [128, NDC, E] partition-major
        return np.ascontiguousarray(
            w.T.astype(BF).reshape(NDC, 128, E).transpose(1, 0, 2))

    wslices = []
    for gi in range(4):
        e0 = E * gi
        wslices.append({
            "wq": _warr(W_q[e0:e0 + E] / 8.0),
            "wk": _warr(W_k[e0:e0 + E]),
            "wv": _warr(W_v[e0:e0 + E]),
            "wu": _warr(W_u[e0:e0 + E]),
            "wo": np.ascontiguousarray(W_o[:, e0:e0 + E].T),
        })

    in_maps = []
    for c in range(8):
        b, gi = c // 4, c % 4
        xtb, xpb, dmk, bias = per_batch[b]
        in_maps.append({
            "xt": xtb, "xp": xpb, "dmask": dmk, "biasd": bias, **wslices[gi],
        })
    return in_maps


def kernel(x, token_types, seq_lens, W_q, W_k, W_v, W_u, W_o, **_run_kwargs):
    token_types = np.asarray(token_types)
    seq_lens = np.asarray(seq_lens)
    plan = _plan(token_types, seq_lens)
    NCH, n_total, n_dense, elo, ehi, _ = plan
    key = (NCH, n_total, n_dense, elo, ehi)
    if _cache.get("key") != key:
        _cache["nc"] = _build(NCH, n_total, n_dense, elo, ehi)
        _cache["key"] = key
    nc = _cache["nc"]
    in_maps = _host_inputs(x, token_types, seq_lens, W_q, W_k, W_v, W_u, W_o,
                           plan)
    try:
        res = run_bass_kernel_spmd(nc, in_maps, list(range(8)), **_run_kwargs)
    except Exception as ex:  # transient NRT device wedge: retry once
        if "UNRECOVERABLE" not in str(ex) and "UNAVAILABLE" not in str(ex):
            raise
        res = run_bass_kernel_spmd(nc, in_maps, list(range(8)), **_run_kwargs)
    _cache["last_result"] = res
    full = np.zeros((B, L, D), np.float64)
    for c in range(8):
        full[c // 4] += res.results[c]["out"].astype(np.float64)
        full[c // 4][L - 512:] += res.results[c]["out2"].astype(np.float64)
    return full.astype(np.float32)
